# revision 7
# baseline (speedup 1.0000x reference)
"""Trainium2 Bass kernel for DescartesExtension (order-2, with replacement).

out[b, k] = x[b, ii[k]] * x[b, jj[k]] with (ii, jj) = triu_indices(D), i.e.
the output row is the concatenation over i of x[b, i] * x[b, i:D].

Sharding: data-parallel over the batch dim — 1024 rows / 8 cores = 128 rows
per core (one SBUF partition tile).

The problem is HBM-write bound: 538 MB of output vs 2 MB of input. All 8
cores together saturate device HBM (~2.9 TB/s), so the fp32 floor is
~180 us. The harness tolerance is rel_err < 2e-2 while bf16 rounding costs
~2.2e-3, so the kernel computes products in fp32/bf16 and stores the output
row in bf16 — halving HBM traffic — and the host upcasts after gathering.

With bf16 output, compute becomes the co-bottleneck. Measured HW costs:
  DVE tensor_scalar bf16->bf16:  232 + 0.234*L ns   (4x mode)
  ACT activation-Copy f32->bf16: 427 + 0.543*L ns   (conversion free)
  (DVE with fp32 src + bf16 dst runs ~14 ns/col — never mix widths;
   Pool/GpSimd multiply ~14 ns/col Q7 software — unusable.)
The 232/427 ns fixed costs dominate for short segments, so segments with
L < 256 are computed 16-at-a-time by a single DVE tensor_tensor with a
sliding-window access pattern (in0 = x[b, i0+g+t], strides [1,1]) times a
broadcast pattern (in1 = x[b, i0+g], stride 0 on t), each group padded to
its max segment length. The ~1.5% padded columns are DMAd and stripped on
the host. Long segments are split greedily between DVE tensor_scalar and
ACT activation to balance finish times.

Output DMAs ride the SP HWDGE ring in ~16K-column chunks (single ring
measured faster than alternating rings; 32 KB/partition descriptors).
"""

import numpy as np

N_CORES = 8
B = 1024
D = 512
K = D * (D + 1) // 2  # 131328
BS = B // N_CORES  # 128 rows per core = one partition tile

# Segments i < TAIL_START are per-segment ops; the rest run as grouped
# sliding-window tensor_tensor ops, GROUP segments per instruction.
TAIL_START = 257  # L = D - i < 256
GROUP = 16
XPAD = GROUP  # window reads up to D + GROUP - 2; pad x tiles

RAMP_UP = [512, 4096]
STEADY_TARGET = 15872  # bf16: 31744-byte rows, single descriptor per row
STEADY_BUFS = 3

# Measured per-instruction costs (ns) for the engine balance.
CV_FIX, CV_COL = 232.0, 0.234  # DVE tensor_scalar bf16
CA_FIX, CA_COL = 427.0, 0.543  # ACT activation-Copy
CT_FIX, CT_COL = 232.0, 1.04  # DVE tensor_tensor grouped (1x mode)

_CACHE = {}


def _plan():
    """Head segments, tail groups, padded layout and chunk list.

    Returns (head_chunks, tail_chunks, groups, k_pad) where
      head_chunks: (seg_start, seg_end, out_off, cols)
      groups:      (i0, g, Lp, out_off)  padded to g*Lp columns
      tail_chunks: (grp_start, grp_end, out_off, cols)
    """
    lengths = [D - i for i in range(D)]
    offs = [0]
    for ln in lengths:
        offs.append(offs[-1] + ln)

    head_cols = offs[TAIL_START]
    # head chunks: ramp then steady
    targets = list(RAMP_UP)
    head_chunks = []
    i = 0
    off = 0
    while i < TAIL_START:
        target = targets.pop(0) if targets else STEADY_TARGET
        s = i
        clen = 0
        while i < TAIL_START and clen < target:
            clen += lengths[i]
            i += 1
        head_chunks.append((s, i, off, clen))
        off += clen
    assert off == head_cols

    # tail groups, padded
    groups = []
    i = TAIL_START
    while i < D:
        g = min(GROUP, D - i)
        Lp = lengths[i]
        groups.append((i, g, Lp, off))
        off += g * Lp
        i += g
    k_pad = off

    # tail chunks: consecutive groups up to ~STEADY_TARGET cols
    tail_chunks = []
    gs = 0
    while gs < len(groups):
        ge = gs
        clen = 0
        while ge < len(groups) and clen < STEADY_TARGET:
            _, g, Lp, _ = groups[ge]
            clen += g * Lp
            ge += 1
        tail_chunks.append((gs, ge, groups[gs][3], clen))
        gs = ge

    return lengths, offs, head_chunks, tail_chunks, groups, k_pad


def _issue_order(n_head, n_tail, n_ramp):
    """Ramp first, then steady head front-to-back with tail chunks (slow to
    produce, DVE-only) interleaved mid-stream between fast head chunks."""
    head = list(range(n_ramp, n_head))
    tail = list(range(n_head, n_head + n_tail))
    order = list(range(n_ramp))
    # place a tail chunk after every second head chunk, starting early so
    # the DVE tensor_tensor work overlaps ACT-heavy head chunks
    hi = 0
    ti = 0
    while hi < len(head) or ti < len(tail):
        for _ in range(2):
            if hi < len(head):
                order.append(head[hi])
                hi += 1
        if ti < len(tail):
            order.append(tail[ti])
            ti += 1
    return order


def _engine_split(lengths, head_chunks, order, n_head, groups, tail_chunks):
    """Greedy balance of per-segment ops between DVE and ACT in issue order;
    tail groups are pinned to DVE (tensor_tensor)."""
    t_v = 0.0
    t_a = 0.0
    assign = {}
    for ci in order:
        if ci >= n_head:
            gs, ge, _off, _cl = tail_chunks[ci - n_head]
            for gi in range(gs, ge):
                _i0, g, Lp, _o = groups[gi]
                t_v += CT_FIX + CT_COL * g * Lp
            continue
        s, e, _off0, _clen = head_chunks[ci]
        for i in range(s, e):
            ln = lengths[i]
            c_v = CV_FIX + CV_COL * ln
            c_a = CA_FIX + CA_COL * ln
            if ci == 0:
                # pinned to ACT: reads fp32 x directly while xt16 is built
                assign[i] = "a"
                t_a += c_a
                continue
            if t_v + c_v <= t_a + c_a:
                assign[i] = "v"
                t_v = t_v + c_v
            else:
                assign[i] = "a"
                t_a = t_a + c_a
    return assign


def _build():
    if "nc" in _CACHE:
        return _CACHE["nc"]
    from bass_rust import AP
    import concourse.tile as tile
    from concourse import bacc, mybir

    lengths, offs, head_chunks, tail_chunks, groups, k_pad = _plan()
    n_head = len(head_chunks)
    n_ramp = len(RAMP_UP)
    order = _issue_order(n_head, len(tail_chunks), n_ramp)
    assign = _engine_split(lengths, head_chunks, order, n_head, groups, tail_chunks)

    nc = bacc.Bacc("TRN2", debug=False)
    x_ap = nc.dram_tensor("x", [BS, D], mybir.dt.float32, kind="ExternalInput").ap()
    out_ap = nc.dram_tensor(
        "out", [BS, k_pad], mybir.dt.bfloat16, kind="ExternalOutput"
    ).ap()

    ramp_max = max(c[3] for c in head_chunks[:n_ramp])
    steady_max = max(
        max(c[3] for c in head_chunks[n_ramp:]),
        max(c[3] for c in tail_chunks),
    )
    XW = D + XPAD

    with tile.TileContext(nc) as tc:
        with (
            tc.tile_pool(name="xp", bufs=1) as xp,
            tc.tile_pool(name="wp", bufs=1) as wp,
            tc.tile_pool(name="rp", bufs=n_ramp + 1) as rp,
            tc.tile_pool(name="op", bufs=STEADY_BUFS) as op,
        ):
            # Pre-warm the ACT activation table concurrently with the x load.
            warm = wp.tile([BS, 2], mybir.dt.float32)
            nc.vector.memset(warm[:], 0.0)
            nc.scalar.activation(
                warm[:], warm[:], mybir.ActivationFunctionType.Copy, scale=1.0
            )

            xt = xp.tile([BS, D], mybir.dt.float32)
            nc.sync.dma_start(xt[:], x_ap[:])
            # bf16 x copy for the DVE paths, padded so sliding windows stay
            # in-bounds; the pad columns are zeroed.
            xt16 = xp.tile([BS, XW], mybir.dt.bfloat16)
            nc.vector.memset(xt16[:, D:XW], 0.0)
            nc.scalar.copy(xt16[:, 0:D], xt[:])
            x16 = xt16[:]  # base AP for raw window patterns

            def win_ap(col_off, d1, s1, d2, s2):
                return AP(
                    x16.tensor, col_off, [[XW, 128], [s1, d1], [s2, d2]]
                )

            for ci in order:
                if ci < n_ramp:
                    ot = rp.tile([BS, ramp_max], mybir.dt.bfloat16, tag="ramp")
                else:
                    ot = op.tile([BS, steady_max], mybir.dt.bfloat16, tag="out")
                o_base = ot[:]

                if ci >= n_head:  # tail chunk: grouped tensor_tensor on DVE
                    gs, ge, off0, clen = tail_chunks[ci - n_head]
                    for gi in range(gs, ge):
                        i0, g, Lp, goff = groups[gi]
                        dst = AP(
                            o_base.tensor,
                            o_base.offset + (goff - off0),
                            [[steady_max, 128], [Lp, g], [1, Lp]],
                        )
                        src = win_ap(i0, g, 1, Lp, 1)  # x[b, i0+g'+t]
                        fac = win_ap(i0, g, 1, Lp, 0)  # x[b, i0+g'] bcast
                        nc.vector.tensor_tensor(
                            dst, src, fac, mybir.AluOpType.mult
                        )
                else:
                    s, e, off0, clen = head_chunks[ci]
                    for i in range(s, e):
                        ln = lengths[i]
                        dst = ot[:, offs[i] - off0 : offs[i] - off0 + ln]
                        scal = xt[:, i : i + 1]
                        if assign[i] == "v":
                            nc.vector.tensor_scalar_mul(
                                dst, xt16[:, i : i + ln], scal
                            )
                        else:
                            nc.scalar.activation(
                                dst,
                                xt[:, i:D],
                                mybir.ActivationFunctionType.Copy,
                                scale=scal,
                            )
                nc.sync.dma_start(out_ap[:, off0 : off0 + clen], ot[:, :clen])

    nc.compile()
    _CACHE["nc"] = nc
    _CACHE["plan"] = (lengths, offs, groups, k_pad)
    return nc


def _bf16_to_f32(a):
    """Exact bf16 -> fp32 upcast via bit manipulation (fast in numpy)."""
    u = a.view(np.uint16).astype(np.uint32) << 16
    return u.view(np.float32)


def _unpad(padded):
    """[B, k_pad] padded bf16 -> [B, K] fp32."""
    lengths, offs, groups, k_pad = _CACHE["plan"]
    out = np.empty((B, K), dtype=np.float32)
    head_cols = offs[TAIL_START]
    out[:, :head_cols] = _bf16_to_f32(np.ascontiguousarray(padded[:, :head_cols]))
    for i0, g, Lp, goff in groups:
        blk = _bf16_to_f32(
            np.ascontiguousarray(padded[:, goff : goff + g * Lp])
        ).reshape(B, g, Lp)
        for gg in range(g):
            i = i0 + gg
            ln = lengths[i]
            out[:, offs[i] : offs[i] + ln] = blk[:, gg, :ln]
    return out


def _run(x, trace=False):
    from concourse.bass_utils import run_bass_kernel_spmd

    nc = _build()
    x = np.ascontiguousarray(x, dtype=np.float32)
    assert x.shape == (B, D), x.shape
    in_maps = [{"x": x[c * BS : (c + 1) * BS]} for c in range(N_CORES)]
    res = run_bass_kernel_spmd(nc, in_maps, list(range(N_CORES)), trace=trace)
    padded = np.concatenate([res.results[c]["out"] for c in range(N_CORES)], axis=0)
    out = _unpad(padded)
    return out, res


def kernel(x):
    return _run(x)[0]


# revision 11
# speedup vs baseline: 1.0256x; 1.0256x over previous
"""Trainium2 Bass kernel for DescartesExtension (order-2, with replacement).

out[b, k] = x[b, ii[k]] * x[b, jj[k]] with (ii, jj) = triu_indices(D), i.e.
the output row is the concatenation over i of x[b, i] * x[b, i:D].

Sharding: data-parallel over the batch dim — 1024 rows / 8 cores = 128 rows
per core (one SBUF partition tile).

The problem is HBM-write bound: 538 MB of output vs 2 MB of input. All 8
cores together saturate device HBM (~2.9 TB/s), so the fp32 floor is
~180 us. The harness tolerance is rel_err < 2e-2 while bf16 rounding costs
~2.2e-3, so the kernel computes products in fp32/bf16 and stores the output
row in bf16 — halving HBM traffic — and the host upcasts after gathering.

With bf16 output, compute becomes the co-bottleneck. Measured HW costs:
  DVE tensor_scalar bf16->bf16:  232 + 0.234*L ns   (4x mode)
  ACT activation-Copy f32->bf16: 427 + 0.543*L ns   (conversion free)
  (DVE with fp32 src + bf16 dst runs ~14 ns/col — never mix widths;
   Pool/GpSimd multiply ~14 ns/col Q7 software — unusable.)
The 232/427 ns fixed costs dominate for short segments, so segments with
L < 256 are computed 16-at-a-time by a single DVE tensor_tensor with a
sliding-window access pattern (in0 = x[b, i0+g+t], strides [1,1]) times a
broadcast pattern (in1 = x[b, i0+g], stride 0 on t), each group padded to
its max segment length. The ~1.5% padded columns are DMAd and stripped on
the host. Long segments are split greedily between DVE tensor_scalar and
ACT activation to balance finish times.

Output DMAs ride the SP HWDGE ring in ~16K-column chunks (single ring
measured faster than alternating rings; 32 KB/partition descriptors).
"""

import numpy as np

N_CORES = 8
B = 1024
D = 512
K = D * (D + 1) // 2  # 131328
BS = B // N_CORES  # 128 rows per core = one partition tile

# Segments i < TAIL_START are per-segment ops; the rest run as grouped
# sliding-window tensor_tensor ops, GROUP segments per instruction.
TAIL_START = 257  # L = D - i < 256
GROUP = 16
XPAD = GROUP  # window reads up to D + GROUP - 2; pad x tiles

RAMP_UP = [512, 1024, 2048, 4096]
STEADY_TARGET = 7936  # bf16: ~16KB rows, good SDMA descriptors, fine pipeline
TAIL_TARGET = 4000  # tail pieces: ~1 group per DMA chunk
STEADY_BUFS = 5

# Measured per-instruction costs (ns) for the engine balance.
CV_FIX, CV_COL = 232.0, 0.234  # DVE tensor_scalar bf16
CA_FIX, CA_COL = 427.0, 0.543  # ACT activation-Copy
CT_FIX, CT_COL = 150.0, 1.04  # DVE tensor_tensor grouped (1x mode)

_CACHE = {}


def _plan():
    """Head segments, tail groups, padded layout and chunk list.

    Returns (head_chunks, tail_chunks, groups, k_pad) where
      head_chunks: (seg_start, seg_end, out_off, cols)
      groups:      (i0, g, Lp, out_off)  padded to g*Lp columns
      tail_chunks: (grp_start, grp_end, out_off, cols)
    """
    lengths = [D - i for i in range(D)]
    offs = [0]
    for ln in lengths:
        offs.append(offs[-1] + ln)

    head_cols = offs[TAIL_START]
    # head chunks: ramp then steady
    targets = list(RAMP_UP)
    head_chunks = []
    i = 0
    off = 0
    while i < TAIL_START:
        target = targets.pop(0) if targets else STEADY_TARGET
        s = i
        clen = 0
        while i < TAIL_START and clen < target:
            clen += lengths[i]
            i += 1
        head_chunks.append((s, i, off, clen))
        off += clen
    assert off == head_cols

    # tail groups, padded
    groups = []
    i = TAIL_START
    while i < D:
        g = min(GROUP, D - i)
        Lp = lengths[i]
        groups.append((i, g, Lp, off))
        off += g * Lp
        i += g
    k_pad = off

    # tail chunks: consecutive groups up to ~TAIL_TARGET cols
    tail_chunks = []
    gs = 0
    while gs < len(groups):
        ge = gs
        clen = 0
        while ge < len(groups) and clen < TAIL_TARGET:
            _, g, Lp, _ = groups[ge]
            clen += g * Lp
            ge += 1
        tail_chunks.append((gs, ge, groups[gs][3], clen))
        gs = ge

    return lengths, offs, head_chunks, tail_chunks, groups, k_pad


def _issue_order(head_chunks, tail_chunks, n_ramp):
    """Ramp first, then steady head chunks with tail chunks interleaved in
    proportion to their column counts, so DVE tensor_tensor work (tail) is
    spread evenly across the ACT-heavy head stream."""
    n_head = len(head_chunks)
    head = list(range(n_ramp, n_head))
    tail = list(range(n_head, n_head + len(tail_chunks)))
    head_cols = sum(head_chunks[i][3] for i in head)
    tail_cols = sum(c[3] for c in tail_chunks)
    order = list(range(n_ramp))
    hi = ti = 0
    hc = tc_ = 0
    while hi < len(head) or ti < len(tail):
        # pick the stream that is behind its proportional share
        if ti >= len(tail) or (
            hi < len(head) and hc * tail_cols <= tc_ * head_cols
        ):
            order.append(head[hi])
            hc += head_chunks[head[hi]][3]
            hi += 1
        else:
            order.append(tail[ti])
            tc_ += tail_chunks[ti][3]
            ti += 1
    return order


def _engine_split(lengths, head_chunks, order, n_head, groups, tail_chunks):
    """Greedy balance of per-segment ops between DVE and ACT in issue order.

    Tail groups are pinned to DVE (tensor_tensor); their total cost is
    pre-charged to DVE's clock so the head greedy compensates by loading
    ACT correspondingly more."""
    t_v = sum(CT_FIX + CT_COL * g * Lp for _i0, g, Lp, _o in groups)
    t_a = 0.0
    assign = {}
    for ci in order:
        if ci >= n_head:
            continue
        s, e, _off0, _clen = head_chunks[ci]
        for i in range(s, e):
            ln = lengths[i]
            c_v = CV_FIX + CV_COL * ln
            c_a = CA_FIX + CA_COL * ln
            if ci == 0:
                # pinned to ACT: reads fp32 x directly while xt16 is built
                assign[i] = "a"
                t_a += c_a
                continue
            if t_v + c_v <= t_a + c_a:
                assign[i] = "v"
                t_v = t_v + c_v
            else:
                assign[i] = "a"
                t_a = t_a + c_a
    return assign


def _build():
    if "nc" in _CACHE:
        return _CACHE["nc"]
    from bass_rust import AP
    import concourse.tile as tile
    from concourse import bacc, mybir

    lengths, offs, head_chunks, tail_chunks, groups, k_pad = _plan()
    n_head = len(head_chunks)
    n_ramp = len(RAMP_UP)
    order = _issue_order(head_chunks, tail_chunks, n_ramp)
    assign = _engine_split(lengths, head_chunks, order, n_head, groups, tail_chunks)

    nc = bacc.Bacc("TRN2", debug=False)
    x_ap = nc.dram_tensor("x", [BS, D], mybir.dt.float32, kind="ExternalInput").ap()
    out_ap = nc.dram_tensor(
        "out", [BS, k_pad], mybir.dt.bfloat16, kind="ExternalOutput"
    ).ap()

    ramp_max = max(c[3] for c in head_chunks[:n_ramp])
    steady_max = max(
        max(c[3] for c in head_chunks[n_ramp:]),
        max(c[3] for c in tail_chunks),
    )
    XW = D + XPAD

    with tile.TileContext(nc) as tc:
        with (
            tc.tile_pool(name="xp", bufs=1) as xp,
            tc.tile_pool(name="wp", bufs=1) as wp,
            tc.tile_pool(name="rp", bufs=n_ramp + 1) as rp,
            tc.tile_pool(name="op", bufs=STEADY_BUFS) as op,
        ):
            # Pre-warm the ACT activation table concurrently with the x load.
            warm = wp.tile([BS, 2], mybir.dt.float32)
            nc.vector.memset(warm[:], 0.0)
            nc.scalar.activation(
                warm[:], warm[:], mybir.ActivationFunctionType.Copy, scale=1.0
            )

            xt = xp.tile([BS, D], mybir.dt.float32)
            nc.sync.dma_start(xt[:], x_ap[:])
            # bf16 x copy for the DVE paths, padded so sliding windows stay
            # in-bounds; the pad columns are zeroed.
            xt16 = xp.tile([BS, XW], mybir.dt.bfloat16)
            nc.vector.memset(xt16[:, D:XW], 0.0)
            nc.scalar.copy(xt16[:, 0:D], xt[:])
            x16 = xt16[:]  # base AP for raw window patterns

            def win_ap(col_off, d1, s1, d2, s2):
                return AP(
                    x16.tensor, col_off, [[XW, 128], [s1, d1], [s2, d2]]
                )

            for ci in order:
                if ci < n_ramp:
                    ot = rp.tile([BS, ramp_max], mybir.dt.bfloat16, tag="ramp")
                else:
                    ot = op.tile([BS, steady_max], mybir.dt.bfloat16, tag="out")
                o_base = ot[:]

                if ci >= n_head:  # tail chunk: grouped tensor_tensor on DVE
                    gs, ge, off0, clen = tail_chunks[ci - n_head]
                    for gi in range(gs, ge):
                        i0, g, Lp, goff = groups[gi]
                        dst = AP(
                            o_base.tensor,
                            o_base.offset + (goff - off0),
                            [[steady_max, 128], [Lp, g], [1, Lp]],
                        )
                        src = win_ap(i0, g, 1, Lp, 1)  # x[b, i0+g'+t]
                        fac = win_ap(i0, g, 1, Lp, 0)  # x[b, i0+g'] bcast
                        nc.vector.tensor_tensor(
                            dst, src, fac, mybir.AluOpType.mult
                        )
                else:
                    s, e, off0, clen = head_chunks[ci]
                    for i in range(s, e):
                        ln = lengths[i]
                        dst = ot[:, offs[i] - off0 : offs[i] - off0 + ln]
                        scal = xt[:, i : i + 1]
                        if assign[i] == "v":
                            nc.vector.tensor_scalar_mul(
                                dst, xt16[:, i : i + ln], scal
                            )
                        else:
                            nc.scalar.activation(
                                dst,
                                xt[:, i:D],
                                mybir.ActivationFunctionType.Copy,
                                scale=scal,
                            )
                nc.sync.dma_start(out_ap[:, off0 : off0 + clen], ot[:, :clen])

    nc.compile()
    _CACHE["nc"] = nc
    _CACHE["plan"] = (lengths, offs, groups, k_pad)
    return nc


def _bf16_to_f32(a):
    """Exact bf16 -> fp32 upcast via bit manipulation (fast in numpy)."""
    u = a.view(np.uint16).astype(np.uint32) << 16
    return u.view(np.float32)


def _unpad(padded):
    """[B, k_pad] padded bf16 -> [B, K] fp32."""
    lengths, offs, groups, k_pad = _CACHE["plan"]
    out = np.empty((B, K), dtype=np.float32)
    head_cols = offs[TAIL_START]
    out[:, :head_cols] = _bf16_to_f32(np.ascontiguousarray(padded[:, :head_cols]))
    for i0, g, Lp, goff in groups:
        blk = _bf16_to_f32(
            np.ascontiguousarray(padded[:, goff : goff + g * Lp])
        ).reshape(B, g, Lp)
        for gg in range(g):
            i = i0 + gg
            ln = lengths[i]
            out[:, offs[i] : offs[i] + ln] = blk[:, gg, :ln]
    return out


def _run(x, trace=False):
    from concourse.bass_utils import run_bass_kernel_spmd

    nc = _build()
    x = np.ascontiguousarray(x, dtype=np.float32)
    assert x.shape == (B, D), x.shape
    in_maps = [{"x": x[c * BS : (c + 1) * BS]} for c in range(N_CORES)]
    res = run_bass_kernel_spmd(nc, in_maps, list(range(N_CORES)), trace=trace)
    padded = np.concatenate([res.results[c]["out"] for c in range(N_CORES)], axis=0)
    out = _unpad(padded)
    return out, res


def kernel(x):
    return _run(x)[0]


# revision 12
# speedup vs baseline: 1.0568x; 1.0305x over previous
"""Trainium2 Bass kernel for DescartesExtension (order-2, with replacement).

out[b, k] = x[b, ii[k]] * x[b, jj[k]] with (ii, jj) = triu_indices(D), i.e.
the output row is the concatenation over i of x[b, i] * x[b, i:D].

Sharding: data-parallel over the batch dim — 1024 rows / 8 cores = 128 rows
per core (one SBUF partition tile).

The problem is HBM-write bound: 538 MB of output vs 2 MB of input. All 8
cores together saturate device HBM (~2.9 TB/s), so the fp32 floor is
~180 us. The harness tolerance is rel_err < 2e-2 while bf16 rounding costs
~2.3e-3, so the kernel computes products in fp32/bf16 and stores the output
row in bf16 — halving HBM traffic — and the host upcasts after gathering.

With bf16 output, compute becomes the co-bottleneck. Measured HW costs:
  DVE tensor_scalar bf16->bf16:    232 + 0.234*L ns  (4x mode)
  DVE tensor_tensor bf16 windowed: 150 + 1.04*L ns   (1x; bcast stride-0)
  ACT activation-Copy f32->bf16:   427 + 0.543*L ns  (conversion free)
  (DVE with fp32 src + bf16 dst runs ~14 ns/col; Pool/GpSimd multiply
   ~14 ns/col Q7 software — both unusable.)

Structure: segments with L >= 256 are per-segment ops; shorter segments are
computed 16-at-a-time by a single DVE tensor_tensor using a sliding-window
access pattern times a stride-0 broadcast pattern, each group padded to its
max length (~1.5% extra columns, stripped on the host).

A two-clock scheduler assigns contiguous runs of work to DVE or ACT so
both engines finish together, and emits each run as ONE single-engine
chunk. The output layout follows chunk-creation (production) order, so the
single SP HWDGE ring (FIFO) drains chunks in exactly the order they
complete — no cross-engine coupling, no head-of-line stalls. The host
reassembles the true column order from the recorded per-segment offsets.
"""

import numpy as np

N_CORES = 8
B = 1024
D = 512
K = D * (D + 1) // 2  # 131328
BS = B // N_CORES  # 128 rows per core = one partition tile

TAIL_START = 257  # segments i >= TAIL_START (L < 256) run as grouped TT
GROUP = 16
XPAD = GROUP  # window reads up to D + GROUP - 2; pad x tiles

RUN_NS = 3500.0  # target per-chunk engine time
MAX_CHUNK_COLS = 8192  # 16 KB bf16 rows: single descriptor per partition
BUFS_PER_ENGINE = 4

# Measured per-instruction costs (ns).
CV_FIX, CV_COL = 232.0, 0.234  # DVE tensor_scalar bf16
CA_FIX, CA_COL = 427.0, 0.543  # ACT activation-Copy
CT_FIX, CT_COL = 150.0, 1.04  # DVE tensor_tensor grouped

_CACHE = {}


def _schedule():
    """Two-clock scheduler: contiguous single-engine chunks in production
    order; output offsets assigned in the same order.

    Returns (chunks, seg_off, grp_off, k_pad):
      chunks: (engine, items, off0, cols); items: ("s", i, L, rel_off) or
              ("g", i0, g, Lp, rel_off)
      seg_off[i]: column offset of head segment i in the padded layout
      grp_off:    list of (i0, g, Lp, off) for tail groups
    """
    lengths = [D - i for i in range(D)]
    head = list(range(TAIL_START))
    groups = []
    i = TAIL_START
    while i < D:
        g = min(GROUP, D - i)
        groups.append((i, g, lengths[i]))
        i += g

    # total tail cost on DVE, to pace tail-group consumption
    tail_cost = sum(CT_FIX + CT_COL * g * Lp for _i, g, Lp in groups)

    chunks = []
    seg_off = {}
    grp_off = []
    t_v = 0.0
    t_a = 0.0
    off = 0
    hi = 0  # next head segment
    gi = 0  # next tail group
    v_tail_turn = False
    while hi < len(head) or gi < len(groups):
        if hi < len(head) and (t_a <= t_v or (gi >= len(groups) and False)):
            eng = "a" if t_a <= t_v else "v"
        else:
            eng = "v" if gi < len(groups) or hi < len(head) else "a"
        items = []
        cols = 0
        run = 0.0
        if eng == "v":
            take_tail = gi < len(groups) and (v_tail_turn or hi >= len(head))
            v_tail_turn = not take_tail
            if take_tail:
                i0, g, Lp = groups[gi]
                gi += 1
                items.append(("g", i0, g, Lp, 0))
                grp_off.append((i0, g, Lp, off))
                cols = g * Lp
                run = CT_FIX + CT_COL * cols
                t_v += run
            else:
                while (
                    hi < len(head)
                    and run < RUN_NS
                    and cols + lengths[hi] <= MAX_CHUNK_COLS
                ):
                    L = lengths[hi]
                    items.append(("s", hi, L, cols))
                    seg_off[hi] = off + cols
                    cols += L
                    run += CV_FIX + CV_COL * L
                    hi += 1
                t_v += run
        else:
            while (
                hi < len(head)
                and run < RUN_NS
                and cols + lengths[hi] <= MAX_CHUNK_COLS
            ):
                L = lengths[hi]
                items.append(("s", hi, L, cols))
                seg_off[hi] = off + cols
                cols += L
                run += CA_FIX + CA_COL * L
                hi += 1
            t_a += run
        if not items:
            continue
        chunks.append((eng, items, off, cols))
        off += cols
    return chunks, seg_off, grp_off, off, t_v, t_a


def _build():
    if "nc" in _CACHE:
        return _CACHE["nc"]
    from bass_rust import AP
    import concourse.tile as tile
    from concourse import bacc, mybir

    chunks, seg_off, grp_off, k_pad, t_v, t_a = _schedule()

    nc = bacc.Bacc("TRN2", debug=False)
    x_ap = nc.dram_tensor("x", [BS, D], mybir.dt.float32, kind="ExternalInput").ap()
    out_ap = nc.dram_tensor(
        "out", [BS, k_pad], mybir.dt.bfloat16, kind="ExternalOutput"
    ).ap()

    XW = D + XPAD
    vmax = max(c[3] for c in chunks if c[0] == "v")
    amax = max(c[3] for c in chunks if c[0] == "a")

    with tile.TileContext(nc) as tc:
        with (
            tc.tile_pool(name="xp", bufs=1) as xp,
            tc.tile_pool(name="wp", bufs=1) as wp,
            tc.tile_pool(name="vp", bufs=BUFS_PER_ENGINE) as vp,
            tc.tile_pool(name="ap_", bufs=BUFS_PER_ENGINE) as ap_,
        ):
            # Pre-warm the ACT activation table concurrently with the x load.
            warm = wp.tile([BS, 2], mybir.dt.float32)
            nc.vector.memset(warm[:], 0.0)
            nc.scalar.activation(
                warm[:], warm[:], mybir.ActivationFunctionType.Copy, scale=1.0
            )

            xt = xp.tile([BS, D], mybir.dt.float32)
            nc.sync.dma_start(xt[:], x_ap[:])
            # bf16 x copy for the DVE paths (ACT converts at full speed),
            # padded so sliding windows stay in-bounds.
            xt16 = xp.tile([BS, XW], mybir.dt.bfloat16)
            nc.vector.memset(xt16[:, D:XW], 0.0)
            nc.scalar.copy(xt16[:, 0:D], xt[:])
            x16 = xt16[:]

            for eng, items, off0, cols in chunks:
                if eng == "v":
                    ot = vp.tile([BS, vmax], mybir.dt.bfloat16, tag="vout")
                else:
                    ot = ap_.tile([BS, amax], mybir.dt.bfloat16, tag="aout")
                o_base = ot[:]
                for it in items:
                    if it[0] == "s":
                        _, i, L, rel = it
                        dst = ot[:, rel : rel + L]
                        scal = xt[:, i : i + 1]
                        if eng == "v":
                            nc.vector.tensor_scalar_mul(
                                dst, xt16[:, i : i + L], scal
                            )
                        else:
                            nc.scalar.activation(
                                dst,
                                xt[:, i : i + L],
                                mybir.ActivationFunctionType.Copy,
                                scale=scal,
                            )
                    else:
                        _, i0, g, Lp, rel = it
                        dst = AP(
                            o_base.tensor,
                            o_base.offset + rel,
                            [[vmax, 128], [Lp, g], [1, Lp]],
                        )
                        src = AP(x16.tensor, i0, [[XW, 128], [1, g], [1, Lp]])
                        fac = AP(x16.tensor, i0, [[XW, 128], [1, g], [0, Lp]])
                        nc.vector.tensor_tensor(
                            dst, src, fac, mybir.AluOpType.mult
                        )
                nc.sync.dma_start(out_ap[:, off0 : off0 + cols], ot[:, :cols])

    nc.compile()
    _CACHE["nc"] = nc
    _CACHE["plan"] = (seg_off, grp_off, k_pad)
    return nc


def _bf16_to_f32(a):
    """Exact bf16 -> fp32 upcast via bit manipulation (fast in numpy)."""
    u = a.view(np.uint16).astype(np.uint32) << 16
    return u.view(np.float32)


def _unpad(padded):
    """[B, k_pad] production-order bf16 -> [B, K] true-order fp32."""
    seg_off, grp_off, k_pad = _CACHE["plan"]
    lengths = [D - i for i in range(D)]
    offs = [0]
    for ln in lengths:
        offs.append(offs[-1] + ln)
    # gather map: true column k -> padded column
    if "idx" not in _CACHE:
        idx = np.empty(K, dtype=np.int64)
        for i in range(TAIL_START):
            o = seg_off[i]
            idx[offs[i] : offs[i] + lengths[i]] = np.arange(o, o + lengths[i])
        for i0, g, Lp, off in grp_off:
            for gg in range(g):
                i = i0 + gg
                ln = lengths[i]
                o = off + gg * Lp
                idx[offs[i] : offs[i] + ln] = np.arange(o, o + ln)
        _CACHE["idx"] = idx
    idx = _CACHE["idx"]
    u16 = padded.view(np.uint16)[:, idx]
    return (u16.astype(np.uint32) << 16).view(np.float32)


def _run(x, trace=False):
    from concourse.bass_utils import run_bass_kernel_spmd

    nc = _build()
    x = np.ascontiguousarray(x, dtype=np.float32)
    assert x.shape == (B, D), x.shape
    in_maps = [{"x": x[c * BS : (c + 1) * BS]} for c in range(N_CORES)]
    res = run_bass_kernel_spmd(nc, in_maps, list(range(N_CORES)), trace=trace)
    padded = np.concatenate([res.results[c]["out"] for c in range(N_CORES)], axis=0)
    out = _unpad(padded)
    return out, res


def kernel(x):
    return _run(x)[0]
